# revision 1
# baseline (speedup 1.0000x reference)
"""TransformerXL attention (AttentionXL) Bass kernel for Trainium2, 8 NeuronCores.

Sharding: pure data-parallel over batch (BS=8 -> 1 batch element per core).
All weights replicated per core; no collectives.

Per-core algorithm (everything bf16 on the PE, fp32 PSUM accumulation):
  Host prep:  X^T, Xc^T, Pos^T (transposed activations so every matmul's
              lhsT/rhs operands are naturally laid out), W_kv split into
              W_k/W_v, bias folds:
                bias_qu = b_q + u.ravel()      (per-channel, q-side)
                bias_qv = b_q + v.ravel()
                b_out   = b_v @ W_proj + b_proj  (exact: softmax rows sum to 1)
  Device:
    KT = W_k^T @ X^T   [hd, j]   (+b_k per-partition)
    RT = W_pos^T @ P^T [hd, m]   (+b_pos)
    QT = W_q^T @ Xc^T  [hd, i]   -> QuT (+bias_qu), QvT (+bias_qv)
    V  = X^T.T @ W_v   [j, hd]
    per head h:
      C  [i,j] = QuT_h^T KT_h           (PE, contraction d=64, row-packed pairs)
      P  [i,m] = QvT_h^T RT_h
      P -> DRAM (contig);  S [i,j] read back with the rel_shift flat trick:
           S[i,j] = Pflat[i*1023 + 511 + j]   (one strided DMA per i-block)
      L = C + S (DVE, from PSUM);  causal mask on j>=512 half (affine_select)
      A = exp(L*0.125) with accum_out -> Z (ScalarE);  A *= 1/Z (DVE)
      A^T via TensorE transpose (128x128 blocks) -> SBUF
      O^T_h [d, i] = V_h^T A^T (PE, col-packed head pairs) -> AVT chunk
    out[i,e] = AVT^T @ W_proj + b_out (ones-row bias matmul), fp32.
"""

import os
import sys

for _p in (
    "/root/.axon_site",
    "/root/.axon_site/_ro/trn_rl_repo",
    "/root/.axon_site/_ro/pypackages",
    "/opt/trn_rl_repo",
):
    if os.path.isdir(_p) and _p not in sys.path:
        sys.path.append(_p)

import numpy as np
import ml_dtypes

import concourse.bass as bass
import concourse.mybir as mybir
import concourse.tile as tile
from concourse.bass_utils import run_bass_kernel_spmd
from concourse.masks import make_identity

BF16 = mybir.dt.bfloat16
FP32 = mybir.dt.float32
AF = mybir.ActivationFunctionType
ALU = mybir.AluOpType
nbf16 = ml_dtypes.bfloat16

CUR, FULL, BS, DIM, H, D = 512, 1024, 8, 1024, 16, 64
PREV = FULL - CUR
SCALE = 1.0 / D**0.5
P = 128
NIB = CUR // P    # 4 query blocks
NJC = FULL // P   # 8 key chunks
NCH = DIM // P    # 8 dim chunks
NHP = H // 2      # 8 head pairs
MASK_FILL = -30000.0

_BUILT = None


def _split_multiwait(nc):
    """walrus here encodes at most ONE sync wait per TPB instruction
    (NEURON_ISA_TPB_EVENTS has a single wait slot).  Split every
    multi-wait instruction: prepend same-engine NoOps carrying the
    extra waits, keep the last wait on the instruction itself."""
    n_split = 0
    for fn in nc.m.functions:
        for blk in fn.blocks:
            insts = list(blk.instructions)
            out = []
            for ins in insts:
                si = ins.sync_info
                if si is not None and si.on_wait and len(si.on_wait) > 1:
                    waits = list(si.on_wait)
                    for w in waits[:-1]:
                        nop = mybir.InstNoOp(
                            name=f"{ins.name}-ws{n_split}",
                            engine=ins.engine,
                            sync_info=mybir.SyncInfo(on_wait=[w], on_update=[]),
                            text_hint="waitsplit",
                        )
                        out.append(nop)
                        n_split += 1
                    ins.sync_info = mybir.SyncInfo(
                        on_wait=[waits[-1]],
                        on_update=list(si.on_update or []),
                    )
                out.append(ins)
            blk.instructions = out
    return n_split


def _build(split_waits=True):
    nc = bass.Bass()

    # acts: [X^T | Xc^T | Pos^T] cols; wmats: [W_q | W_pos | W_k | W_v] cols
    acts = nc.declare_dram_parameter("acts", [DIM, FULL + CUR + FULL], BF16, isOutput=False)
    wmats = nc.declare_dram_parameter("wmats", [DIM, 4 * DIM], BF16, isOutput=False)
    wproj = nc.declare_dram_parameter("wproj", [DIM, DIM], BF16, isOutput=False)
    # biases pre-laid-out on host: [p, 4*NCH] = qu | qv | k | pos chunks
    biases = nc.declare_dram_parameter("biases", [P, 4 * NCH], FP32, isOutput=False)
    bout = nc.declare_dram_parameter("bout", [DIM], BF16, isOutput=False)
    out = nc.declare_dram_parameter("out", [CUR, DIM], FP32, isOutput=True)

    with tile.TileContext(nc) as tc:
        from contextlib import ExitStack

        with ExitStack() as ctx:
            persist = ctx.enter_context(tc.tile_pool(name="persist", bufs=1))

            KT = persist.tile([P, NCH, FULL], BF16, tag="KT")
            RT = persist.tile([P, NCH, FULL], BF16, tag="RT")
            V = persist.tile([P, NJC, DIM], BF16, tag="V")
            QuT = persist.tile([P, NCH, CUR], BF16, tag="QuT")
            QvT = persist.tile([P, NCH, CUR], BF16, tag="QvT")
            AVT = persist.tile([P, NCH, CUR], BF16, tag="AVT")
            ones_row = persist.tile([P, P], BF16, tag="ones_row")
            bout_t = persist.tile([P, DIM], BF16, tag="bout_t")
            bias_t = persist.tile([P, 4, NCH], FP32, tag="bias_t")  # qu|qv|k|pos

            ident = persist.tile([P, P], BF16, tag="ident")
            make_identity(nc, ident)
            mask_fill_reg = nc.gpsimd.to_reg(MASK_FILL)
            nc.vector.memset(ones_row, 0.0)
            nc.vector.memset(ones_row[0:1, :], 1.0)
            nc.vector.memset(bout_t, 0.0)
            nc.sync.dma_start(bout_t[0:1, :], bout[None, :])
            nc.sync.dma_start(bias_t, biases.rearrange("p (b c) -> p b c", b=4))

            # ---------------- Stage A: projections ----------------
            with tc.tile_pool(name="ain", bufs=1) as ain, tc.tile_pool(
                name="apsum", bufs=4, space="PSUM"
            ) as apsum:
                acts_t = ain.tile([P, NCH, FULL + CUR + FULL], BF16, tag="acts")
                wmats_t = ain.tile([P, NCH, 4 * DIM], BF16, tag="wmats")
                nc.sync.dma_start(acts_t, acts.rearrange("(c p) f -> p c f", p=P))
                nc.sync.dma_start(wmats_t, wmats.rearrange("(c p) f -> p c f", p=P))
                xT_t = acts_t[:, :, 0:FULL]
                xcT_t = acts_t[:, :, FULL : FULL + CUR]
                pT_t = acts_t[:, :, FULL + CUR : FULL + CUR + FULL]
                wq_t = wmats_t[:, :, 0:DIM]
                wpos_t = wmats_t[:, :, DIM : 2 * DIM]
                wk_t = wmats_t[:, :, 2 * DIM : 3 * DIM]
                wv_t = wmats_t[:, :, 3 * DIM : 4 * DIM]

                # per-engine observer copies: absorb DMA-lane waits early so no
                # downstream instruction exceeds the ISA sync-wait limit
                dmy = ain.tile([P, 16], FP32, tag="dmy")
                col = [0]
                def _observe(eng):
                    for srcap in (acts_t[:, 0, 0:2], wmats_t[:, 0, 0:2],
                                  bias_t[:, 0, 0:2], bout_t[:, 0:2]):
                        eng(dmy[:, col[0] : col[0] + 2], srcap)
                        col[0] = (col[0] + 2) % 16
                _observe(nc.vector.tensor_copy)
                _observe(nc.scalar.copy)

                # Q^T [hd, i] then QuT/QvT with per-partition bias
                for oc in range(NCH):
                    ps = apsum.tile([P, CUR], FP32, tag="aps")
                    for kc in range(NCH):
                        nc.tensor.matmul(
                            ps,
                            wq_t[:, kc, oc * P : (oc + 1) * P],
                            xcT_t[:, kc, :],
                            start=(kc == 0),
                            stop=(kc == NCH - 1),
                        )
                    nc.scalar.activation(
                        QuT[:, oc, :], ps, AF.Identity, bias=bias_t[:, 0, oc : oc + 1]
                    )
                    nc.scalar.activation(
                        QvT[:, oc, :], ps, AF.Identity, bias=bias_t[:, 1, oc : oc + 1]
                    )

                # K^T [hd, j] and R^T [hd, m]
                for oc in range(NCH):
                    for jh in range(2):
                        sl = slice(jh * 512, (jh + 1) * 512)
                        ps = apsum.tile([P, 512], FP32, tag="aps2")
                        for kc in range(NCH):
                            nc.tensor.matmul(
                                ps,
                                wk_t[:, kc, oc * P : (oc + 1) * P],
                                xT_t[:, kc, sl],
                                start=(kc == 0),
                                stop=(kc == NCH - 1),
                            )
                        nc.scalar.activation(
                            KT[:, oc, sl], ps, AF.Identity,
                            bias=bias_t[:, 2, oc : oc + 1],
                        )
                        ps = apsum.tile([P, 512], FP32, tag="aps2")
                        for kc in range(NCH):
                            nc.tensor.matmul(
                                ps,
                                wpos_t[:, kc, oc * P : (oc + 1) * P],
                                pT_t[:, kc, sl],
                                start=(kc == 0),
                                stop=(kc == NCH - 1),
                            )
                        nc.scalar.activation(
                            RT[:, oc, sl], ps, AF.Identity,
                            bias=bias_t[:, 3, oc : oc + 1],
                        )

                # V [j, hd]
                for jc in range(NJC):
                    for mh in range(2):
                        sl = slice(mh * 512, (mh + 1) * 512)
                        ps = apsum.tile([P, 512], FP32, tag="aps2")
                        for kc in range(NCH):
                            nc.tensor.matmul(
                                ps,
                                xT_t[:, kc, jc * P : (jc + 1) * P],
                                wv_t[:, kc, sl],
                                start=(kc == 0),
                                stop=(kc == NCH - 1),
                            )
                        nc.vector.tensor_copy(V[:, jc, sl], ps)

            # ---------------- Stage B: attention per head ----------------
            late = ctx.enter_context(tc.tile_pool(name="late", bufs=1))
            work = ctx.enter_context(tc.tile_pool(name="work", bufs=3))
            pswork = ctx.enter_context(tc.tile_pool(name="pswork", bufs=2))
            ahead = ctx.enter_context(tc.tile_pool(name="ahead", bufs=2))
            dram = ctx.enter_context(tc.tile_pool(name="dram", bufs=4, space="DRAM"))
            cps = ctx.enter_context(tc.tile_pool(name="cps", bufs=2, space="PSUM"))
            pps = ctx.enter_context(tc.tile_pool(name="pps", bufs=2, space="PSUM"))
            tps = ctx.enter_context(tc.tile_pool(name="tps", bufs=2, space="PSUM"))
            avp = ctx.enter_context(tc.tile_pool(name="avp", bufs=1, space="PSUM"))

            WPROJ = late.tile([P, NCH, DIM], BF16, tag="WPROJ")
            nc.sync.dma_start(WPROJ, wproj.rearrange("(c p) f -> p c f", p=P))

            for hp in range(NHP):
                at_pair = []
                for hh in range(2):
                    h = 2 * hp + hh
                    ch, ro = divmod(h, 2)
                    ro *= D
                    rs = slice(ro, ro + D)

                    # --- position scores P [i, m] -> DRAM (one 1MB DMA) ---
                    # row i only needs m >= 511 - i; per block: m >= 384-128*ib
                    p_all = pswork.tile([P, NIB, FULL], BF16, tag="p_all")
                    for ib in range(NIB):
                        isl = slice(ib * P, (ib + 1) * P)
                        mlo = 0  # full m-range: the rel-shift wrap reads low m of
                        # the next row, so trimming creates undefined DRAM reads
                        for mh in range(2):
                            m0, m1 = mh * 512, (mh + 1) * 512
                            if m1 <= mlo:
                                continue
                            m0 = max(m0, mlo)
                            pp = pps.tile([P, 512], FP32, tag="pp")
                            w = m1 - m0
                            nc.tensor.matmul(
                                pp[:, :w], QvT[rs, ch, isl], RT[rs, ch, m0:m1],
                                start=True, stop=True,
                            )
                            nc.scalar.copy(p_all[:, ib, m0:m1], pp[:, :w])
                    pdram = dram.tile([CUR, FULL], BF16, tag="pdram")
                    nc.sync.dma_start(
                        pdram.rearrange("(ib p) m -> p ib m", p=P), p_all
                    )
                    # shifted read, all blocks in one DMA:
                    # S[ib*128+u, j] = Pflat[(ib*128+u)*1023 + 511 + j]
                    s_all = pswork.tile([P, NIB, FULL], BF16, tag="s_all")
                    sh_ap = bass.AP(
                        tensor=pdram.tensor,
                        offset=pdram.offset + (PREV - 1),
                        ap=[[FULL - 1, P], [(FULL - 1) * P, NIB], [1, FULL]],
                    )
                    nc.sync.dma_start(s_all, sh_ap)

                    # --- per i-block: C + S, mask, softmax, transpose ---
                    a_t = ahead.tile([P, NJC, CUR], BF16, tag="at")  # A^T [j, i]
                    at_pair.append(a_t)
                    for ib in range(NIB):
                        isl = slice(ib * P, (ib + 1) * P)
                        jmax = 640 + ib * P                  # valid j < jmax
                        l_sb = work.tile([P, FULL], BF16, tag="l_sb")
                        for jh in range(2):
                            j0, j1 = jh * 512, min((jh + 1) * 512, jmax)
                            w = j1 - j0
                            cp = cps.tile([P, 512], FP32, tag="cp")
                            nc.tensor.matmul(
                                cp[:, :w], QuT[rs, ch, isl], KT[rs, ch, j0:j1],
                                start=True, stop=True,
                            )
                            nc.vector.tensor_tensor(
                                l_sb[:, j0:j1], cp[:, :w], s_all[:, ib, j0:j1],
                                ALU.add,
                            )
                        # causal mask on j in [512, jmax): valid iff (i0+u)-j' >= 0
                        nc.gpsimd.affine_select(
                            out=l_sb[:, 512:jmax],
                            in_=l_sb[:, 512:jmax],
                            compare_op=ALU.is_ge,
                            fill=mask_fill_reg,
                            base=ib * P,
                            channel_multiplier=1,
                            pattern=[[-1, jmax - 512]],
                        )
                        # exp + row sums
                        a_sb = work.tile([P, FULL], BF16, tag="a_sb")
                        z_t = work.tile([P, 1], FP32, tag="z_t")
                        nc.scalar.activation(
                            a_sb[:, :jmax], l_sb[:, :jmax], AF.Exp,
                            scale=SCALE, accum_out=z_t,
                        )
                        rz = work.tile([P, 1], FP32, tag="rz")
                        nc.vector.reciprocal(rz, z_t)
                        nc.vector.tensor_scalar_mul(
                            a_sb[:, :jmax], a_sb[:, :jmax], rz
                        )

                        # transpose valid A blocks on PE, 4 per psum tile
                        njc_v = min(ib + 5, NJC)
                        for tg in range(2):
                            jcs = [j for j in range(tg * 4, min((tg + 1) * 4, njc_v))]
                            if not jcs:
                                continue
                            tp = tps.tile([P, 4, P], BF16, tag="tp")
                            for k, jc in enumerate(jcs):
                                nc.tensor.transpose(
                                    tp[:, k], a_sb[:, jc * P : (jc + 1) * P], ident
                                )
                            nc.vector.tensor_copy(
                                a_t[:, jcs[0] : jcs[0] + len(jcs), isl],
                                tp[:, : len(jcs)],
                            )

                # --- AV for the head pair: O^T [d, i], col-packed ---
                av2 = [avp.tile([P, CUR], FP32, tag="av_a", name="av_a"),
                       avp.tile([P, CUR], FP32, tag="av_b", name="av_b")]
                for jc in range(NJC):
                    ilo = max(0, (jc - 4)) * P
                    for hh in range(2):
                        h = 2 * hp + hh
                        nc.tensor.matmul(
                            av2[hh][hh * D : (hh + 1) * D, ilo:],
                            V[:, jc, h * D : (h + 1) * D],
                            at_pair[hh][:, jc, ilo:],
                            start=(jc == 0),
                            stop=(jc == NJC - 1),
                            tile_position=(0, hh * D),
                        )
                nc.vector.tensor_copy(AVT[0:D, hp, :], av2[0][0:D, :])
                nc.vector.tensor_copy(AVT[D:P, hp, :], av2[1][D:P, :])

            # ---------------- Final projection ----------------
            with tc.tile_pool(name="fin", bufs=1) as fin:
                o_all = fin.tile([P, NIB, DIM], FP32, tag="o_all")
                for ib in range(NIB):
                    isl = slice(ib * P, (ib + 1) * P)
                    for eh in range(2):
                        esl = slice(eh * 512, (eh + 1) * 512)
                        fp = avp.tile([P, 512], FP32, tag="av_a")
                        for fc in range(NCH):
                            nc.tensor.matmul(
                                fp, AVT[:, fc, isl], WPROJ[:, fc, esl],
                                start=(fc == 0), stop=False,
                            )
                        nc.tensor.matmul(
                            fp, ones_row, bout_t[:, esl], start=False, stop=True
                        )
                        nc.vector.tensor_copy(o_all[:, ib, esl], fp)
                nc.sync.dma_start(out.rearrange("(ib p) e -> p ib e", p=P), o_all)

    if split_waits:
        _split_multiwait(nc)
    return nc


def _get_nc():
    global _BUILT
    if _BUILT is None:
        _BUILT = _build()
    return _BUILT


def _prep_host(inputs, pos_embedding, full_input, u, v, mask,
               W_kv, b_kv, W_q, b_q, W_pos, b_pos, W_proj, b_proj):
    f32 = np.float32
    W_k = np.ascontiguousarray(W_kv[:, : H * D])
    W_v = np.ascontiguousarray(W_kv[:, H * D :])
    b_k = b_kv[: H * D].astype(f32)
    b_v = b_kv[H * D :].astype(f32)
    bias_qu = (b_q + u.ravel()).astype(f32)
    bias_qv = (b_q + v.ravel()).astype(f32)
    b_out = (b_v @ W_proj + b_proj).astype(f32)

    bias_all = np.stack(
        [bias_qu.reshape(NCH, P), bias_qv.reshape(NCH, P),
         b_k.reshape(NCH, P), b_pos.astype(f32).reshape(NCH, P)], axis=0
    )  # [4, NCH, P]
    bias_all = np.ascontiguousarray(bias_all.transpose(2, 0, 1).reshape(P, 4 * NCH))
    wmats_np = np.concatenate([W_q, W_pos, W_k, W_v], axis=1).astype(nbf16)
    shared = {
        "wmats": wmats_np,
        "wproj": W_proj.astype(nbf16),
        "biases": bias_all.astype(f32),
        "bout": b_out.astype(nbf16),
    }
    pT_np = pos_embedding[:, 0].T
    in_maps = []
    for c in range(BS):
        m = dict(shared)
        m["acts"] = np.concatenate(
            [full_input[:, c].T, inputs[:, c].T, pT_np], axis=1
        ).astype(nbf16)
        in_maps.append(m)
    return in_maps


def kernel(**inputs):
    nc = _get_nc()
    in_maps = _prep_host(**{k: np.asarray(v) for k, v in inputs.items()})
    res = run_bass_kernel_spmd(nc, in_maps, list(range(BS)))
    out = np.stack([res.results[c]["out"] for c in range(BS)], axis=1)
    return np.ascontiguousarray(out.astype(np.float32))


if __name__ == "__main__":
    nc = _build()
    print("built ok")

